# revision 22
# baseline (speedup 1.0000x reference)
"""Trainium2 Bass kernel for nn_Attention_49907519980190 (v2).

Reference computation (b=2, n=2048, dim=1024, h=16, d=64):
    q = (x @ w_q)   -> (b, h, n, d)
    k, v = split(x @ w_vk)
    dots = (q @ k^T) * sqrt(d)          # NOTE: multiplies by 8
    attn = softmax(dots)
    out = (attn @ v) reassembled -> (b, n, h*d) @ w_out

Sharding (8 cores): batch x head-group parallel. Core c handles batch
b = c // 4 and heads 4*(c % 4) .. 4*(c % 4) + 4. Column-parallel
q/k/v projections, row-parallel out projection; the host sums the four
partial outputs per batch.

v2 design (vs baseline):
- Single-pass fp16 projections (emulated rel err 0.0045 vs 2e-2 gate;
  the bf16 hi/lo 3-pass of the baseline was overkill).
- x^T produced by fp16 cast + DMA transpose (sync queue) instead of
  128 PE transposes + ACT copies.
- Q^T/K^T stored pair-stacked: partition = d-of-head-pair (head 2m in
  rows 0:64, head 2m+1 in 64:128): full-width PSUM evacuations and
  K=64 dots matmuls (auto tile_position from base partition).
- Softmax max via ONE DVE tensor_tensor_reduce per (h, it):
  bias8 = min((Sh0 max Sh1) * -8) = -8 * rowmax. Full max (subset max
  is numerically catastrophic here: logit sigma ~64).
- exp on ACT in two [128, 1024] instructions per (h, it) with
  accum_out giving the softmax denominator for free.
- PV reoriented: V tile as stationary ([128 k, 64]), P^T as moving
  (N=512), producing O^T[d, q] directly in PSUM (col-tiled per head
  pair: head 2m -> out partitions 0:64, 2m+1 -> 64:128). Kills the
  baseline's 1024 LDW-bound N=65 matmuls and the phase-E PE
  transposes. Denominator reciprocals are transposed to a row via a
  small DMA gather, partition-broadcast on gpsimd, and applied in one
  DVE multiply per pair during PSUM evacuation.
- PSUM: dots S in [128, 1024] halves (2 banks, bufs=2) + O^T pair
  accumulator [128, 2048] (4 banks) = 8 banks exactly.
"""

import numpy as np

import concourse.bass as bass
import concourse.mybir as mybir
import concourse.tile as tile
from concourse import bacc
from concourse.bass_utils import run_bass_kernel_spmd

F32 = mybir.dt.float32
BF16 = mybir.dt.bfloat16
FP16 = mybir.dt.float16
ADD = mybir.AluOpType.add
MULT = mybir.AluOpType.mult
MAX = mybir.AluOpType.max
MIN = mybir.AluOpType.min
AX = mybir.AxisListType.X
EXP = mybir.ActivationFunctionType.Exp

P = 128      # partitions
NTOK = 2048  # tokens per core (one batch slice)
DIM = 1024   # model dim
E = 256      # per-core projection width (4 heads x 64)
NH = 4       # heads per core
D = 64       # head dim
KO = 8       # contraction chunks of 128 over DIM
TT = 16      # token tiles of 128
SCALE = 8.0  # sqrt(D); reference MULTIPLIES by it
FLT_BIG = 3.0e38


def build_attention_nc():
    nc = bacc.Bacc("TRN2", target_bir_lowering=False, debug=False)

    x = nc.declare_dram_parameter("x", [NTOK, DIM], F32, isOutput=False)
    wq = nc.declare_dram_parameter("wq", [DIM, E], F32, isOutput=False)
    wk = nc.declare_dram_parameter("wk", [DIM, E], F32, isOutput=False)
    wv = nc.declare_dram_parameter("wv", [DIM, E], F32, isOutput=False)
    wo = nc.declare_dram_parameter("wo", [E, DIM], F32, isOutput=False)
    y = nc.declare_dram_parameter("y", [NTOK, DIM], F32, isOutput=True)

    with tile.TileContext(nc) as tc:
        with tc.tile_pool(name="persist", bufs=1) as persist:
            # Q^T / K^T pair-stacked: partition = d of head pair
            # (head 2m rows 0:64, head 2m+1 rows 64:128), free = (pair, tok)
            QT2 = persist.tile([P, 2, NTOK], FP16)
            KT2 = persist.tile([P, 2, NTOK], FP16)
            # V natural: [tok_low, tok_tile, e]
            Vb = persist.tile([P, TT, E], FP16)
            wo16 = persist.tile([P, 2, DIM], FP16)
            # O^T pair-stacked fp16: partition = e of pair, free = (pair, tok)
            OT = persist.tile([P, 2, NTOK], FP16)
            # unnormalized O^T, evacuated per 512-token group
            OTraw = persist.tile([P, 2, NTOK], FP16)
            # softmax denominators per (head, it), q on partitions
            den4 = persist.tile([P, NH, TT], F32)
            # reciprocal rows: partition hh holds head hh's 1/denom over q
            recT = persist.tile([P, NTOK], FP16)

            # ---------- Phase A: weights + x^T (cast + DMA transpose)
            with tc.tile_pool(name="xw", bufs=1) as xw:
                xT = xw.tile([P, KO, NTOK], FP16)  # x^T: [d_low, d_chunk, tok]
                wq16 = xw.tile([P, KO, E], FP16)
                wk16 = xw.tile([P, KO, E], FP16)
                wv16 = xw.tile([P, KO, E], FP16)

                with tc.tile_pool(name="stage", bufs=1) as stage:
                    for wsrc, wdst in ((wk, wk16), (wq, wq16), (wv, wv16)):
                        wf = stage.tile([P, KO, E], F32, tag="wf", bufs=2)
                        nc.scalar.dma_start(
                            out=wf,
                            in_=wsrc[:, :].rearrange("(ko p) e -> p ko e", p=P),
                        )
                        nc.scalar.copy(out=wdst, in_=wf)
                    wof = stage.tile([P, 2, DIM], F32, tag="wof", bufs=1)
                    nc.scalar.dma_start(
                        out=wof, in_=wo[:, :].rearrange("(eo p) d -> p eo d", p=P)
                    )
                    nc.scalar.copy(out=wo16, in_=wof)

                    for tt in range(TT):
                        ts = slice(tt * P, (tt + 1) * P)
                        xf = stage.tile([P, DIM], F32, tag="xf", bufs=3)
                        ldq = nc.gpsimd if tt % 2 == 0 else nc.scalar
                        ldq.dma_start(out=xf, in_=x[ts, :])
                        xc = stage.tile([P, DIM], FP16, tag="xc", bufs=3)
                        ceng = nc.vector if tt % 2 == 0 else nc.gpsimd
                        ceng.tensor_copy(out=xc, in_=xf)
                        nc.sync.dma_start_transpose(out=xT[:, :, ts], in_=xc)

                    # ---------- Phase C: projections (single-pass fp16)
                    with tc.tile_pool(name="psA", bufs=1, space="PSUM") as psA:
                        # K then Q for pair 0 first, V in between, pair 1 last
                        def proj(w16, dst, m, scale=1.0):
                            ms = slice(m * P, (m + 1) * P)
                            pr4 = psA.tile([P, 4, 512], F32, tag="pr4", bufs=1)
                            for g in range(4):
                                for c in range(KO):
                                    nc.tensor.matmul(
                                        pr4[:, g, :], w16[:, c, ms],
                                        xT[:, c, g * 512:(g + 1) * 512],
                                        start=(c == 0), stop=(c == KO - 1),
                                    )
                            # Q is pre-scaled by 8 so the dots come out as the
                            # final logits and the exp bias is just -rowmax
                            nc.scalar.mul(
                                out=dst[:, m, :],
                                in_=pr4.rearrange("p g n -> p (g n)"),
                                mul=scale,
                            )

                        proj(wk16, KT2, 0)
                        proj(wq16, QT2, 0, scale=SCALE)
                        for tm in range(TT):
                            tms = slice(tm * P, (tm + 1) * P)
                            prv = psA.tile([P, E], F32, tag="prv", bufs=2)
                            for c in range(KO):
                                nc.tensor.matmul(
                                    prv, xT[:, c, tms], wv16[:, c, :],
                                    start=(c == 0), stop=(c == KO - 1),
                                )
                            nc.vector.tensor_copy(out=Vb[:, tm, :], in_=prv)
                        proj(wk16, KT2, 1)
                        proj(wq16, QT2, 1, scale=SCALE)

            # ---------- Phase D: attention
            with (
                tc.tile_pool(name="psS", bufs=2, space="PSUM") as psS,
                tc.tile_pool(name="attn_sb", bufs=1) as attn_sb,
                tc.tile_pool(name="attn_small", bufs=1) as attn_small,
            ):
                # PT4[itg]: P^T for 4 q-tiles: [k_low, k_tile, q(512)]
                pt4s = [[None] * 4 for _ in range(NH)]
                recb = [None, None]   # per-pair reciprocal broadcast

                def issue_pv_group(h, itg):
                    row = (h % 2) * D
                    hs = slice(h * D, (h + 1) * D)
                    # share the 8-bank S ring (brief lifetime, half cols)
                    o_g = psS.tile([P, 1024], F32, tag="Sh", bufs=4,
                                   name="o_g")[:, 0:512]
                    for jo in range(TT):
                        nc.tensor.matmul(
                            o_g[row:row + D, :],
                            Vb[:, jo, hs],
                            pt4s[h][itg][:, jo, :],
                            start=(jo == 0), stop=(jo == TT - 1),
                        )
                    nc.vector.tensor_copy(
                        out=OTraw[row:row + D, h // 2,
                                  itg * 512:(itg + 1) * 512],
                        in_=o_g[row:row + D, :],
                    )

                def prep_rec_head(hh):
                    # replicate recT row to 64 partitions by doubling DMAs
                    rb = recb[hh // 2]
                    base = (hh % 2) * D
                    dq = nc.gpsimd if hh % 2 == 0 else nc.scalar
                    dq.dma_start(
                        out=rb[base:base + 1, :], in_=recT[32 * hh:32 * hh + 1, :]
                    )
                    w = 1
                    while w < D:
                        dq.dma_start(
                            out=rb[base + w:base + 2 * w, :],
                            in_=rb[base:base + w, :],
                        )
                        w *= 2

                def evac_pair(pair):
                    nc.vector.tensor_tensor(
                        out=OT[:, pair, :], in0=OTraw[:, pair, :],
                        in1=recb[pair], op=MULT,
                    )

                for h in range(NH):
                    m = h // 2
                    row = (h % 2) * D
                    if h % 2 == 0:
                        recb[m] = attn_sb.tile(
                            [P, NTOK], FP16, tag="recb", bufs=2, name="recb_t"
                        )
                    for it in range(TT):
                        isl = slice(it * P, (it + 1) * P)
                        halves = []
                        for half in range(2):
                            sh = psS.tile([P, 1024], F32, tag="Sh", bufs=4)
                            for nn in range(2):
                                ks = slice(half * 1024 + nn * 512,
                                           half * 1024 + (nn + 1) * 512)
                                nc.tensor.matmul(
                                    sh[:, nn * 512:(nn + 1) * 512],
                                    QT2[row:row + D, m, isl],
                                    KT2[row:row + D, m, ks],
                                    start=True, stop=True,
                                )
                            halves.append(sh)
                        # bias8 = -8 * rowmax via negated per-half maxes
                        # (DVE may read only one PSUM input per instruction)
                        nm01 = attn_small.tile([P, 2], F32, tag="nm01", bufs=6)
                        for half in range(2):
                            # read only the high 16 bits of each fp32 (= bf16
                            # truncation): 2x DVE rate, max off by < 2^-8 rel
                            hi = halves[half].bitcast(BF16).rearrange(
                                "p (n two) -> p two n", two=2
                            )[:, 1:2, :]
                            nc.vector.tensor_reduce(
                                out=nm01[:, half:half + 1], in_=hi,
                                axis=AX, op=MAX, negate=True,
                            )
                        nmn = attn_small.tile([P, 1], F32, tag="nmn", bufs=6)
                        nc.vector.tensor_reduce(
                            out=nmn, in_=nm01, axis=AX, op=MIN,
                        )
                        u_t = attn_sb.tile([P, NTOK], FP16, tag="u", bufs=6)
                        denh = attn_small.tile([P, 2], F32, tag="denh", bufs=4)
                        for half in range(2):
                            nc.scalar.activation(
                                out=u_t[:, half * 1024:(half + 1) * 1024],
                                in_=halves[half], func=EXP,
                                bias=nmn, scale=1.0,
                                accum_out=denh[:, half:half + 1],
                            )
                        nc.vector.tensor_reduce(
                            out=den4[:, h, it:it + 1], in_=denh,
                            axis=AX, op=ADD,
                        )
                        # reciprocal column, then a tiny DMA "transpose"
                        # (partition-major read -> one row of recT)
                        reccol = attn_small.tile([P, 1], FP16, tag="rc", bufs=4)
                        with nc.allow_low_precision(reason="fp16 softmax recip"):
                            nc.vector.reciprocal(
                                out=reccol, in_=den4[:, h, it:it + 1]
                            )
                        rdq = nc.scalar if it % 2 == 0 else nc.gpsimd
                        rdq.dma_start(
                            out=recT[32 * h:32 * h + 1, isl], in_=reccol
                        )
                        itg, itq = it // 4, it % 4
                        if itq == 0:
                            pt4s[h][itg] = attn_sb.tile(
                                [P, TT, 512], FP16, tag="PT4", bufs=6,
                                name="pt4",
                            )
                        nc.sync.dma_start_transpose(
                            out=pt4s[h][itg][:, :, itq * P:(itq + 1) * P],
                            in_=u_t,
                        )
                        # PV for the previous 4-tile q-group (one it of lag so
                        # the PE never stalls on the just-issued transpose)
                        if itq == 0 and itg > 0:
                            issue_pv_group(h, itg - 1)
                    issue_pv_group(h, 3)
                    prep_rec_head(h)
                    if h % 2 == 1:
                        evac_pair(h // 2)

            # ---------- Phase E: y = O @ wo from O^T directly
            with (
                tc.tile_pool(name="psE", bufs=2, space="PSUM") as psE,
                tc.tile_pool(name="ysb", bufs=1) as ysb,
            ):
                for tm in range(TT):
                    ms = slice(tm * P, (tm + 1) * P)
                    for n in range(2):
                        ns = slice(n * 512, (n + 1) * 512)
                        yp = psE.tile([P, 512], F32, tag="yp", bufs=2)
                        for eo in range(2):
                            nc.tensor.matmul(
                                yp[:, :], OT[:, eo, ms], wo16[:, eo, ns],
                                start=(eo == 0), stop=(eo == 1),
                            )
                        yo = ysb.tile([P, 512], F32, tag="yo", bufs=4)
                        nc.vector.tensor_copy(out=yo, in_=yp)
                        eng = nc.sync if (tm + n) % 2 == 0 else nc.scalar
                        eng.dma_start(out=y[ms, ns], in_=yo)

    nc.compile()
    return nc


_NC_CACHE = None


def _get_nc():
    global _NC_CACHE
    if _NC_CACHE is None:
        _NC_CACHE = build_attention_nc()
    return _NC_CACHE


def kernel(x, w_q, w_vk, w_out, **run_kwargs):
    """Full inputs in, full output out. Shards over 8 NeuronCores."""
    b, n, dim = x.shape
    assert (b, n, dim) == (2, 2048, 1024)
    w_k = w_vk[:, :1024]
    w_v = w_vk[:, 1024:]

    in_maps = []
    for c in range(8):
        bi = c // 4
        hg = c % 4
        cs = slice(hg * E, (hg + 1) * E)
        in_maps.append({
            "x": np.ascontiguousarray(x[bi]).astype(np.float32),
            "wq": np.ascontiguousarray(w_q[:, cs]).astype(np.float32),
            "wk": np.ascontiguousarray(w_k[:, cs]).astype(np.float32),
            "wv": np.ascontiguousarray(w_v[:, cs]).astype(np.float32),
            "wo": np.ascontiguousarray(w_out[cs, :]).astype(np.float32),
        })

    nc = _get_nc()
    res = run_bass_kernel_spmd(nc, in_maps, core_ids=list(range(8)), **run_kwargs)
    out = np.zeros((2, 2048, 1024), dtype=np.float32)
    for c in range(8):
        out[c // 4] += res.results[c]["y"]
    if run_kwargs:
        kernel.last_results = res
    return out


# revision 24
# speedup vs baseline: 2.1710x; 2.1710x over previous
"""Trainium2 Bass kernel for nn_Attention_49907519980190.

Reference computation (b=2, n=2048, dim=1024, h=16, d=64):
    q = (x @ w_q)   -> (b, h, n, d)
    k, v = split(x @ w_vk)
    dots = (q @ k^T) * sqrt(d)          # NOTE: multiplies by 8
    attn = softmax(dots)
    out = (attn @ v) reassembled -> (b, n, h*d) @ w_out

Sharding (8 cores): batch x head-group parallel. Core c handles batch
b = c // 4 and heads 4*(c % 4) .. 4*(c % 4) + 4. Column-parallel
q/k/v projections, row-parallel out projection; the host sums the four
partial outputs per batch (the "all-reduce" of row-parallel TP).

Numerics: the softmax logits have std ~75 and the softmax is ~97%
one-hot, so Q/K/dots need fp32-class precision. The PE's in-matmul
accumulator is block-aligned (drops addends ~2^-11 below the running
max) but PSUM accumulation BETWEEN matmuls is exact fp32. So Q, K and
dots use a bf16 hi/lo decomposition: x = hi + lo with both bf16;
a@b ~= ah@bh + (al@bh + ah@bl), a main matmul plus one stacked
correction matmul ([qh;ql] x [kl;kh]) accumulating in PSUM
(measured ~5e-6 matmul rel err vs 2.4e-3 plain bf16).

Softmax: instead of an exact row max (expensive full-width DVE reduce
of PSUM), use a stride-2 subset max m^ and compute
u' = exp((l - 8*m^)/2) in fp32, then u = u'^2 on GPSIMD (exp-squared
doubles the usable logit-gap range to ~176; P(subset max short by
>176) ~ 1e-8 per row). The softmax denominator comes for free as a
65th all-ones column appended per-head to V (the PV matmul then
produces sum_j u_ij in column 64). Attention probabilities and the
whole post-softmax path are fp16.

Schedule: software-pipelined attention loop (PV for iteration N-3
issues between the dots of iteration N), PSUM S-quarters recycle
progressively, projections start as soon as the first token-group of
x^T lands, DMA-transposes all stay on the sync queue (issuing
transposes from two queues concurrently corrupts data - measured).
"""

import numpy as np

import concourse.bass as bass
import concourse.mybir as mybir
import concourse.tile as tile
from concourse import bacc
from concourse.bass_utils import run_bass_kernel_spmd
from concourse.masks import make_identity

F32 = mybir.dt.float32
BF16 = mybir.dt.bfloat16
FP16 = mybir.dt.float16
SUB = mybir.AluOpType.subtract
MULT = mybir.AluOpType.mult
MAX = mybir.AluOpType.max
AX = mybir.AxisListType.X
EXP = mybir.ActivationFunctionType.Exp

P = 128      # partitions
NTOK = 2048  # tokens per core (one batch slice)
DIM = 1024   # model dim
E = 256      # per-core projection width (4 heads x 64)
NH = 4       # heads per core
D = 64       # head dim
D1 = 65      # head dim + ones column (denominator trick)
KO = 8       # contraction chunks of 128 over DIM
TT = 16      # token tiles of 128
NG = 4       # token groups (of 512) for x^T
SCALE = 8.0  # sqrt(D); reference MULTIPLIES by it


def build_attention_nc():
    nc = bacc.Bacc("TRN2", target_bir_lowering=False, debug=False)

    x = nc.declare_dram_parameter("x", [NTOK, DIM], F32, isOutput=False)
    wq = nc.declare_dram_parameter("wq", [DIM, E], F32, isOutput=False)
    wk = nc.declare_dram_parameter("wk", [DIM, E], F32, isOutput=False)
    wv = nc.declare_dram_parameter("wv", [DIM, E], F32, isOutput=False)
    wo = nc.declare_dram_parameter("wo", [E, DIM], F32, isOutput=False)
    y = nc.declare_dram_parameter("y", [NTOK, DIM], F32, isOutput=True)

    with tile.TileContext(nc) as tc:
        with tc.tile_pool(name="persist", bufs=1) as persist:
            # Q^T stacked per head: rows 0:64 = q_hi, 64:128 = q_lo
            # q/k in fp16 (11-bit mantissa): single-pass dots is within
            # the error budget where bf16 was not; rows 64:128 are zeros so
            # the 128-row stationary/moving shapes stay unchanged
            QTs = persist.tile([P, NH, NTOK], FP16)
            KThz = persist.tile([P, NH, NTOK], FP16)
            # V natural [tok_low, tok_hi, head*(64+ones)] fp16
            Vb = persist.tile([P, TT, NH * D1], FP16)
            Ob = persist.tile([P, TT, E], FP16)
            # O^T [emb_low, emb_hi(2), tok]
            OTb = persist.tile([P, 2, NTOK], FP16)
            wob = persist.tile([P, 2, DIM], FP16)
            ident = persist.tile([P, P], FP16)
            make_identity(nc, ident)
            identF = persist.tile([P, P], F32)
            make_identity(nc, identF)
            # ones columns of Vb (written once)
            vb4 = Vb.rearrange("p t (h c) -> p t h c", c=D1)
            nc.vector.memset(vb4[:, :, :, D:D1], 1.0)
            nc.vector.memset(KThz[64:128, :, :], 0.0)
            nc.vector.memset(QTs[64:128, :, :], 0.0)

            with tc.tile_pool(name="xpool", bufs=1) as xpool:
                # x^T fp16 in 4 token-group tiles: [dim_low, dim_hi, 512].
                # Single-pass fp16 projections: emulated end-to-end rel err
                # 0.0045 vs the 2e-2 gate, so the bf16 hi/lo 3-pass is
                # unnecessary PE work (~50us of phase-C matmuls).
                xh = []
                for g in range(NG):
                    xh_g = xpool.tile([P, KO, 512], FP16, tag=f"xh{g}")
                    xh.append(xh_g)
                wqh = xpool.tile([P, KO, E], FP16)
                wkh = xpool.tile([P, KO, E], FP16)
                wvh = xpool.tile([P, KO, E], FP16)

                # ---------- Phase A/B: weights, x split hi/lo + transpose
                with (
                    tc.tile_pool(name="psT", bufs=3, space="PSUM") as psT,
                    tc.tile_pool(name="psA", bufs=3, space="PSUM") as psA,
                ):
                    with tc.tile_pool(name="stage", bufs=4) as stage:

                        for tt in range(TT):
                            g, lt = tt // 4, tt % 4
                            ts = slice(tt * P, (tt + 1) * P)
                            gs = slice(lt * P, (lt + 1) * P)
                            xf = stage.tile([P, DIM], F32, tag="xf")
                            ldq = nc.gpsimd if tt % 2 == 0 else nc.sync
                            ldq.dma_start(out=xf, in_=x[ts, :])
                            for half in range(2):
                                cr = slice(half * 4, half * 4 + 4)
                                pt4 = psT.tile([P, 4, P], F32, tag="pt4")
                                for c4 in range(4):
                                    c = half * 4 + c4
                                    nc.tensor.transpose(
                                        pt4[:, c4, :], xf[:, c * P:(c + 1) * P],
                                        identF[:, :],
                                    )
                                nc.scalar.copy(
                                    out=xh[g][:, cr, gs], in_=pt4
                                )
                        for wsrc, hdst in ((wk, wkh), (wq, wqh)):
                            wf = stage.tile([P, KO, E], F32, tag="wf", bufs=1)
                            nc.sync.dma_start(
                                out=wf,
                                in_=wsrc[:, :].rearrange("(ko p) e -> p ko e", p=P),
                            )
                            nc.scalar.copy(out=hdst, in_=wf)
                        wf = stage.tile([P, KO, E], F32, tag="wf", bufs=1)
                        nc.sync.dma_start(
                            out=wf, in_=wv[:, :].rearrange("(ko p) e -> p ko e", p=P)
                        )
                        nc.scalar.copy(out=wvh, in_=wf)
                        wof = stage.tile([P, 2, DIM], F32, tag="wof", bufs=1)
                        nc.sync.dma_start(
                            out=wof, in_=wo[:, :].rearrange("(eo p) d -> p eo d", p=P)
                        )
                        nc.scalar.copy(out=wob, in_=wof)

                    # ---------- Phase C: projections (single-pass fp16)
                    def proj_fp16(wh, m, g):
                        pr = psA.tile([P, 512], F32, tag="pr")
                        ms = slice(m * P, (m + 1) * P)
                        for c in range(KO):
                            nc.tensor.matmul(
                                pr[:, :], wh[:, c, ms], xh[g][:, c, :],
                                start=(c == 0), stop=(c == KO - 1),
                            )
                        return pr

                    for g in range(NG):
                        ns = slice(g * 512, (g + 1) * 512)
                        for m in range(2):
                            pr = proj_fp16(wkh, m, g)
                            for hh in range(2):
                                h = 2 * m + hh
                                rows = slice(hh * 64, hh * 64 + 64)
                                nc.scalar.copy(
                                    out=KThz[0:64, h, ns], in_=pr[rows, :]
                                )
                        for m in range(2):
                            pr = proj_fp16(wqh, m, g)
                            for hh in range(2):
                                h = 2 * m + hh
                                rows = slice(hh * 64, hh * 64 + 64)
                                nc.scalar.copy(
                                    out=QTs[0:64, h, ns], in_=pr[rows, :]
                                )
                        for lt in range(4):
                            tm = g * 4 + lt
                            pr = psA.tile([P, 512], F32, tag="pr")
                            for c in range(KO):
                                nc.tensor.matmul(
                                    pr[:, :E], xh[g][:, c, lt * P:(lt + 1) * P],
                                    wvh[:, c, :],
                                    start=(c == 0), stop=(c == KO - 1),
                                )
                            for h in range(NH):
                                nc.scalar.copy(
                                    out=Vb[:, tm, h * D1:h * D1 + D],
                                    in_=pr[:, h * D:(h + 1) * D],
                                )

            # ---------- Phase D: attention, software-pipelined depth 3
            with (
                tc.tile_pool(name="psS", bufs=7, space="PSUM") as psS,
                tc.tile_pool(name="psO", bufs=1, space="PSUM") as psO,
                tc.tile_pool(name="attn_sb", bufs=3) as attn_sb,
                tc.tile_pool(name="attn_small", bufs=6) as attn_small,
            ):
                pending = []

                def issue_dots(h, it):
                    isl = slice(it * P, (it + 1) * P)
                    quarters = []
                    for nn in range(4):
                        Sq = psS.tile([P, 512], F32, tag="S")
                        quarters.append(Sq)
                    # single fp16 pass: [q; 0-padded] x [k; 0]
                    for nn in range(4):
                        ns = slice(nn * 512, (nn + 1) * 512)
                        nc.tensor.matmul(
                            quarters[nn][:, :], QTs[:, h, isl], KThz[:, h, ns],
                            start=True, stop=True,
                        )
                    return quarters

                def issue_softmax(h, it, quarters):
                    mx4 = attn_small.tile([P, 4], F32, tag="mx4")
                    for nn in range(4):
                        nc.vector.tensor_reduce(
                            out=mx4[:, nn:nn + 1], in_=quarters[nn], axis=AX, op=MAX
                        )
                    nmx = attn_small.tile([P, 1], F32, tag="nmx")
                    nc.vector.tensor_reduce(
                        out=nmx, in_=mx4, axis=AX, op=MAX, negate=True
                    )
                    bias8 = attn_small.tile([P, 1], F32, tag="bias8")
                    nc.gpsimd.tensor_scalar_mul(bias8, nmx, SCALE)
                    u = attn_sb.tile([P, NTOK], FP16, tag="u", bufs=5)
                    for nn in range(4):
                        cs = slice(nn * 512, (nn + 1) * 512)
                        nc.scalar.activation(
                            out=u[:, cs], in_=quarters[nn], func=EXP,
                            bias=bias8, scale=SCALE,
                        )
                    PT = attn_sb.tile([P, TT, P], FP16, tag="PT", bufs=8)
                    nc.sync.dma_start_transpose(out=PT, in_=u)
                    return PT

                def issue_pv(h, it, PT):
                    O_ps = psO.tile([P, D1], F32, tag="O")
                    for jo in range(TT):
                        nc.tensor.matmul(
                            O_ps[:, :], PT[:, jo, :],
                            Vb[:, jo, h * D1:(h + 1) * D1],
                            start=(jo == 0), stop=(jo == TT - 1),
                        )
                    rec = attn_small.tile([P, 1], F32, tag="rec")
                    nc.vector.reciprocal(out=rec, in_=O_ps[:, D:D1])
                    nc.scalar.activation(
                        out=Ob[:, it, h * D:(h + 1) * D], in_=O_ps[:, :D],
                        func=mybir.ActivationFunctionType.Copy, scale=rec,
                    )

                step = 0
                for h in range(NH):
                    for it in range(TT):
                        quarters = issue_dots(h, it)
                        # batch PV issue in pairs every other iteration so the
                        # dots blocks form long dense PE-array stretches (HAM)
                        if step % 2 == 1:
                            while len(pending) > 4:
                                issue_pv(*pending.pop(0))
                        PT = issue_softmax(h, it, quarters)
                        pending.append((h, it, PT))
                        step += 1
                while pending:
                    issue_pv(*pending.pop(0))

            # ---------- Phase E: O^T then y = O @ wo
            with (
                tc.tile_pool(name="psE", bufs=2, space="PSUM") as psE,
                tc.tile_pool(name="ysb", bufs=3) as ysb,
            ):
                for tt in range(TT):
                    ts = slice(tt * P, (tt + 1) * P)
                    for eo in range(2):
                        pt = psE.tile([P, P], FP16, tag="pt")
                        nc.tensor.transpose(
                            pt[:, :], Ob[:, tt, eo * P:(eo + 1) * P], ident[:, :]
                        )
                        nc.scalar.copy(out=OTb[:, eo, ts], in_=pt)
                for tm in range(TT):
                    ms = slice(tm * P, (tm + 1) * P)
                    for n in range(2):
                        ns = slice(n * 512, (n + 1) * 512)
                        yp = psE.tile([P, 512], F32, tag="yp")
                        for eo in range(2):
                            nc.tensor.matmul(
                                yp[:, :], OTb[:, eo, ms], wob[:, eo, ns],
                                start=(eo == 0), stop=(eo == 1),
                            )
                        yo = ysb.tile([P, 512], F32, tag="yo")
                        nc.vector.tensor_copy(out=yo, in_=yp)
                        eng = nc.sync if (tm + n) % 2 == 0 else nc.scalar
                        eng.dma_start(out=y[ms, ns], in_=yo)

    nc.compile()
    return nc


_NC_CACHE = None


def _get_nc():
    global _NC_CACHE
    if _NC_CACHE is None:
        _NC_CACHE = build_attention_nc()
    return _NC_CACHE


def kernel(x, w_q, w_vk, w_out, **run_kwargs):
    """Full inputs in, full output out. Shards over 8 NeuronCores."""
    b, n, dim = x.shape
    assert (b, n, dim) == (2, 2048, 1024)
    w_k = w_vk[:, :1024]
    w_v = w_vk[:, 1024:]

    in_maps = []
    for c in range(8):
        bi = c // 4
        hg = c % 4
        cs = slice(hg * E, (hg + 1) * E)
        in_maps.append({
            "x": np.ascontiguousarray(x[bi]).astype(np.float32),
            "wq": np.ascontiguousarray(w_q[:, cs]).astype(np.float32),
            "wk": np.ascontiguousarray(w_k[:, cs]).astype(np.float32),
            "wv": np.ascontiguousarray(w_v[:, cs]).astype(np.float32),
            "wo": np.ascontiguousarray(w_out[cs, :]).astype(np.float32),
        })

    nc = _get_nc()
    res = run_bass_kernel_spmd(nc, in_maps, core_ids=list(range(8)), **run_kwargs)
    out = np.zeros((2, 2048, 1024), dtype=np.float32)
    for c in range(8):
        out[c // 4] += res.results[c]["y"]
    if run_kwargs:
        kernel.last_results = res
    return out

